# revision 50
# baseline (speedup 1.0000x reference)
"""Trainium2 Bass kernel for nn_EquiConv2d (equirectangular deformable conv).

Key structural facts exploited (derived from the reference geometry):
  * off_y is exactly longitude-invariant, so each (tap k, row h) samples two
    fixed input rows (iy0, iy0+1) with a constant y-fraction.
  * off_x is longitude-invariant up to the 2*pi wrap: sampling along a row is
    a CIRCULAR shift by a constant s0(k,h) plus a constant x-fraction.
  * Hence the deformable conv is a set of matmul "slots" per output row
    ([128=(c x row-pair) contraction, 512 free]) reading circularly
    duplicated row-pair tiles at per-(k,h) column offsets, with the bilinear
    corner weights folded into the stationary (weight) operand.
  * NEW vs baseline: output rows are processed in PAIRS sharing one PSUM
    bank (top 64 partitions = even row, bottom 64 = odd row).  Slots of the
    two rows that read the SAME moving stream (same input row-pair event,
    same column window, same wrap-seam variant) are MERGED into a single
    matmul with a [128, 128] stationary — the ky-ladder of the equirect
    geometry makes ~30% of all slots mergeable, cutting tensor-engine work
    by the same fraction with bit-identical arithmetic.
  * Two fp32 oddities handled exactly: tap (k=7,h=255) is identically zero
    and tap (k=1,h=1) samples near the antipode with fp32-noise-scattered
    positions -> handled by 3 extra matmul slots with per-column coefficient
    vectors (data-driven, active only on the cores owning global row 1).

Sharding: 8 cores = 2 batches x 4 bands of 32 output-row pairs.
"""

import math

import numpy as np

# ----------------------------------------------------------------------------
# problem constants
B, C, H, W = 2, 64, 256, 512
O, KH, KW = 64, 3, 3
K = KH * KW
NCORES = 8
NROW = 64            # output rows per core
NPAIR = NROW // 2    # output row-pairs per core
NSPEC = 3            # special (antipode) slots, accumulated into local row 1
RING = 32            # staged row-pair ring slots
PF = 2               # staging prefetch lead (row-pairs)
LTLEAD = 5           # lt-table DMA prefetch lead (row-pairs)
SLOTW = 1536         # [A=row | B=row | Z=row w/ col0 zeroed] per ring slot
SKIP_TOL = 1e-4      # drop matmul slots with |scale| below this
FP8T = 0.0           # max |stationary scale| for fp8 DoubleRow slots (0=off)
LTSCALE = 64.0       # stationary pre-scale (fp8 normal range); undone in
                     # the output activation via scale=1/LTSCALE

_CACHE = {}


# ----------------------------------------------------------------------------
# host-side geometry tables (must replicate reference fp32 semantics exactly)

def _compute_offsets_jax():
    """Bit-exact replica of reference.equi_offsets on jax CPU."""
    import jax
    import jax.numpy as jnp
    cpu = jax.devices("cpu")[0]
    with jax.default_device(cpu):
        dtype = jnp.float32
        pano_H, pano_W, kH, kW = H, W, KH, KW
        Kk = kH * kW
        u = jnp.arange(pano_W, dtype=dtype)
        v = jnp.arange(pano_H, dtype=dtype)
        phi = (u - pano_W / 2.0) / pano_W * (2.0 * math.pi)
        theta = -(v - pano_H / 2.0) / pano_H * math.pi
        cp, sp = jnp.cos(phi), jnp.sin(phi)
        z, one = jnp.zeros_like(cp), jnp.ones_like(cp)
        Ry = jnp.stack([jnp.stack([cp, z, sp], -1),
                        jnp.stack([z, one, z], -1),
                        jnp.stack([-sp, z, cp], -1)], -2)
        ct, st = jnp.cos(theta), jnp.sin(theta)
        zh, oh = jnp.zeros_like(ct), jnp.ones_like(ct)
        Rx = jnp.stack([jnp.stack([oh, zh, zh], -1),
                        jnp.stack([zh, ct, -st], -1),
                        jnp.stack([zh, st, ct], -1)], -2)
        ROT = jnp.einsum('wij,hjk->hwik', Ry, Rx)
        fov_w = kW * (2.0 * math.pi / pano_W)
        focal = (kW / 2.0) / math.tan(fov_w / 2.0)
        hg = (jnp.arange(kH, dtype=dtype)[:, None] + 0.5 - kH / 2.0)
        wg = (jnp.arange(kW, dtype=dtype)[None, :] + 0.5 - kW / 2.0)
        hg = jnp.broadcast_to(hg, (kH, kW)).reshape(Kk)
        wg = jnp.broadcast_to(wg, (kH, kW)).reshape(Kk)
        rays0 = jnp.stack([wg / focal, hg / focal, jnp.ones(Kk, dtype)], 0)
        rays0 = rays0 / jnp.linalg.norm(rays0, axis=0, keepdims=True)
        rays = jnp.einsum('hwik,kn->hwin', ROT, rays0)
        phi2 = jnp.arctan2(rays[..., 0, :], rays[..., 2, :])
        th2 = jnp.arcsin(jnp.clip(rays[..., 1, :], -1.0, 1.0))
        x = pano_W / (2.0 * math.pi) * phi2 + pano_W / 2.0
        y = pano_H / math.pi * th2 + pano_H / 2.0
        off_x = x - (wg[None, None, :] + u[None, :, None])
        off_y = y - (hg[None, None, :] + v[:, None, None])
        return (np.asarray(jnp.transpose(off_y, (2, 0, 1))),
                np.asarray(jnp.transpose(off_x, (2, 0, 1))))


def _build_tap_tables():
    off_y, off_x = _compute_offsets_jax()
    ky = np.repeat(np.arange(KH), KW).astype(np.float32)
    kx = np.tile(np.arange(KW), KH).astype(np.float32)
    base_x = (np.arange(W, dtype=np.float32) - np.float32(1))
    base_y = (np.arange(H, dtype=np.float32) - np.float32(1))
    px = (base_x[None, None, :] + kx[:, None, None] + off_x).astype(np.float32)
    py = (base_y[None, :, None] + ky[:, None, None] + off_y).astype(np.float32)
    pyc = py[:, :, 0]
    assert np.all(py == pyc[:, :, None]), "off_y not longitude-invariant"

    iy0 = np.floor(pyc).astype(np.int64)
    wy1 = (pyc - np.floor(pyc)).astype(np.float64)
    v0 = (iy0 >= 0) & (iy0 < H)
    v1 = (iy0 + 1 >= 0) & (iy0 + 1 < H)
    cy0 = np.where(v0, 1.0 - wy1, 0.0)
    cy1 = np.where(v1, wy1, 0.0)

    Draw = np.mod((px.astype(np.float64) - np.arange(W)[None, None, :]), 512.0)
    ang = Draw / 512.0 * 2 * np.pi
    mean = np.mod(np.angle(np.exp(1j * ang).mean(axis=2)) / (2 * np.pi) * 512.0,
                  512.0)
    resid = np.mod(Draw - mean[:, :, None] + 256.0, 512.0) - 256.0
    D = mean + np.median(resid, axis=2)
    s0 = np.mod(np.floor(D), 512).astype(np.int64)
    frac = D - np.floor(D)

    special = np.zeros((K, H), dtype=bool)
    special[1, 1] = True
    dead = (cy0 == 0.0) & (cy1 == 0.0)

    Ddev = np.abs(np.mod(Draw - D[:, :, None] + 256.0, 512.0) - 256.0)
    dev = Ddev.max(axis=2)
    bad = (dev > 5e-4) & ~special & ~dead
    assert not bad.any(), f"unrepresentable taps: {np.argwhere(bad)}"

    def ref_coefs(p):
        x0 = math.floor(p)
        fr = p - x0
        out = {}
        for ix, wt in ((x0, 1.0 - fr), (x0 + 1, fr)):
            if 0 <= ix < W and wt != 0.0:
                out[ix] = out.get(ix, 0.0) + wt
        return out

    # seam variant selection: decided by the exact fp32 px at the wrap column
    slot0_useG = np.zeros((K, H), dtype=bool)
    slot1_useF = np.zeros((K, H), dtype=bool)
    for k in range(K):
        for h in range(H):
            if special[k, h] or dead[k, h]:
                continue
            s = int(s0[k, h]); fr = frac[k, h]
            if s >= 1:
                w0 = (512 - s) % 512
                rc = ref_coefs(float(px[k, h, w0]))
                slot0_useG[k, h] = (abs(rc.get(0, 0.0))
                                    < abs(rc.get(0, 0.0) - (1 - fr)))
            w1 = (511 - s) % 512
            rc = ref_coefs(float(px[k, h, w1]))
            slot1_useF[k, h] = (abs(rc.get(0, 0.0) - fr)
                                < abs(rc.get(0, 0.0)))

    # special tap (1,1): per-column coefficients on F offsets 255..257
    pxs = px[1, 1, :].astype(np.float64)
    Gam = np.zeros((3, W), dtype=np.float64)
    for w in range(W):
        p = pxs[w]
        x0 = math.floor(p)
        fr = p - x0
        for ix, wt in ((x0, 1.0 - fr), (x0 + 1, fr)):
            if 0 <= ix < W and wt != 0.0:
                found = False
                for jj in range(3):
                    if (255 + jj + w) % 512 == ix % 512:
                        Gam[jj, w] += wt
                        found = True
                        break
                assert found, (w, p, ix)

    return dict(iy0=iy0, cy0=cy0, cy1=cy1, s0=s0, frac=frac,
                slot0_useG=slot0_useG, slot1_useF=slot1_useF,
                special=special, dead=dead, Gam=Gam)


# ----------------------------------------------------------------------------
# slot -> (event row, window, variant) keys + merged emit schedule

def _row_slots(tt, h):
    """Slots of output row h: list of (key, k, half_scales).
    key = (input_row, window_col, zeroed_variant);
    scales = (coef_row0, coef_row1) to fold into the stationary halves."""
    out = []
    for k in range(K):
        if tt['dead'][k, h] or tt['special'][k, h]:
            continue
        s = int(tt['s0'][k, h]); fr = float(tt['frac'][k, h])
        iy = int(np.clip(tt['iy0'][k, h], 0, 255))
        c0 = float(tt['cy0'][k, h]); c1 = float(tt['cy1'][k, h])
        cmax = max(abs(c0), abs(c1))
        if (1.0 - fr) * cmax >= SKIP_TOL:
            zer = bool(tt['slot0_useG'][k, h]) and s >= 1
            out.append(((iy, s, zer), k, (c0 * (1 - fr), c1 * (1 - fr))))
        if fr * cmax >= SKIP_TOL:
            zer = not bool(tt['slot1_useF'][k, h])
            out.append(((iy, s + 1, zer), k, (c0 * fr, c1 * fr)))
    return out


def _build_schedule(tt):
    """Per band: event list (input row-pairs, in first-use order over pairs),
    staging targets, and per-pair emit lists.

    emit = dict(ev, win, zer, top=[(k,c0,c1)...], bot=[...], ltcol, width)
    ordered merged-first (within groups by event index)."""
    iy_spc = int(np.clip(tt['iy0'][1, 1], 0, 255))
    blocks = []
    for blk in range(4):
        ev_of, events, first_use = {}, [], []
        pairs = []
        # process polar-heavy pairs (many distinct input rows) LAST so the
        # staging ring warms up on cheap pairs instead of stalling the PE
        order = list(range(NPAIR))
        for it, hp in enumerate(order):
            h0 = blk * NROW + 2 * hp
            h1 = h0 + 1
            if blk == 0 and hp == 0 and iy_spc not in ev_of:
                ev_of[iy_spc] = len(events)
                events.append(iy_spc)
                first_use.append(it)
            keymap = {}
            for (key, k, sc) in _row_slots(tt, h0):
                keymap.setdefault(key, (dict(), dict()))[0].setdefault(
                    k, [0.0, 0.0])
                e = keymap[key][0][k]
                e[0] += sc[0]; e[1] += sc[1]
            for (key, k, sc) in _row_slots(tt, h1):
                keymap.setdefault(key, (dict(), dict()))[1].setdefault(
                    k, [0.0, 0.0])
                e = keymap[key][1][k]
                e[0] += sc[0]; e[1] += sc[1]
            # register events (input rows) in deterministic order
            emits = []
            for key in keymap:
                iy = key[0]
                if iy not in ev_of:
                    ev_of[iy] = len(events)
                    events.append(iy)
                    first_use.append(it)
                top, bot = keymap[key]
                emits.append(dict(ev=ev_of[iy], win=key[1], zer=key[2],
                                  top=top, bot=bot,
                                  merged=bool(top) and bool(bot)))
            # merged first, then solos; within each group by event order
            emits.sort(key=lambda em: (not em['merged'], em['ev'], em['win']))
            for em in emits:
                em['width'] = 128 if em['merged'] else 64
                em['mode'] = 'f16'

            # fp8 DoubleRow pairing: fuse two low-|scale| emits with the
            # same psum region into one half-rate fp8 matmul
            def maxsc(em):
                m = 0.0
                for dd in (em['top'], em['bot']):
                    for _k, (s0, s1) in dd.items():
                        m = max(m, abs(s0), abs(s1))
                return m

            def voff(em):
                off = em['win'] if not em['zer'] else W + em['win']
                return (em['ev'] % RING) * SLOTW + off

            records = []
            if FP8T > 0:
                groups = {}
                for em in emits:
                    if em['merged']:
                        continue   # DoubleRow caps stationary at 2x64 cols
                    reg = 't' if em['top'] else 'b'
                    if maxsc(em) <= FP8T:
                        groups.setdefault(reg, []).append(em)
                for reg, ems in groups.items():
                    ems.sort(key=voff)
                    i = 0
                    while i + 1 < len(ems):
                        a, bb = ems[i], ems[i + 1]
                        if a['width'] == bb['width']:
                            a['mode'] = bb['mode'] = 'f8'
                            records.append(dict(
                                mode='f8', merged=a['merged'],
                                top=a['top'] or bb['top'],
                                bot=a['bot'] or bb['bot'],
                                width=a['width'], parts=(a, bb),
                                ev=min(a['ev'], bb['ev'])))
                            i += 2
                        else:
                            i += 1
            for em in emits:
                if em['mode'] == 'f16':
                    records.append(dict(mode='f16', merged=em['merged'],
                                        top=em['top'], bot=em['bot'],
                                        width=em['width'], parts=(em,),
                                        ev=em['ev']))
            # psum-start safety: region-covering (merged) records first
            records.sort(key=lambda r: (not r['merged'], r['ev']))

            # assign lt columns (f16) and lt8 columns (fp8, 2 blocks each)
            col = col8 = 0
            for r in records:
                if r['mode'] == 'f16':
                    r['ltcol'] = col
                    col += r['width']
                else:
                    r['ltcol'] = col8
                    col8 += 2 * r['width']
            pairs.append(dict(emits=emits, records=records,
                              ltw=col, ltw8=col8, hpair=hp))
        blocks.append(dict(events=events, first_use=first_use, pairs=pairs))

    E = max(len(b['events']) for b in blocks)
    LTW = max(pr['ltw'] for b in blocks for pr in b['pairs'])
    LT8W = max(max(pr['ltw8'] for b in blocks for pr in b['pairs']), 64)
    for b in blocks:
        while len(b['events']) < E:
            b['events'].append(b['events'][-1])

    # staging target per pair: staged-count needed before pair p is
    # tgt[p] = U[min(p+PF, NPAIR-1)] where U = events first-used by <= p
    for b in blocks:
        fu = np.asarray(b['first_use'])
        Uv = np.array([int(np.searchsorted(fu, p, 'right'))
                       for p in range(NPAIR)])
        b['tgt'] = [int(Uv[min(p + PF, NPAIR - 1)]) for p in range(NPAIR)]
        # ring-overwrite feasibility: stage(e) is issued in iteration ls[e]
        # AFTER emitting pair ls[e]-LTLEAD, so every reader of the slot it
        # overwrites (event e-RING) must have been emitted by then.  Matmul
        # emission lags staging by LTLEAD pairs, hence the margin.
        ls = np.full(E, NPAIR, np.int64)
        tgt = np.asarray(b['tgt'])
        for e in range(E):
            hit = np.where(tgt > e)[0]
            if len(hit):
                ls[e] = hit[0]
        lastuse = {}
        for p in range(NPAIR):
            for em in b['pairs'][p]['emits']:
                lastuse[em['ev']] = p
        for e in range(RING, E):
            if e - RING in lastuse:
                assert lastuse[e - RING] <= ls[e] - LTLEAD, \
                    f"RING={RING} too small: ev{e} overwrites ev{e-RING} " \
                    f"(lastuse pair {lastuse[e-RING]}, staged in it " \
                    f"{ls[e]}, emit lag {LTLEAD})"
    return blocks, E, (LTW, LT8W)


# ----------------------------------------------------------------------------
# device program

def _emit_pair_section(tc, aps, tiles, blkinfo, j, ltts0):
    """Emit one per-band section (all-static APs)."""
    import concourse.mybir as mybir
    nc = tc.nc
    f16 = mybir.dt.float16
    f32 = mybir.dt.float32
    f8 = mybir.dt.float8e4
    buf, buf8, coeft, biast, ltst = tiles
    xb, outd, lt = aps['xb'], aps['out'], aps['lt']
    xb8, lt8 = aps['xb8'], aps['lt8']
    psp, ltp, lt8p, zp, outp = tiles_pools[0]
    tgt = blkinfo['tgt']
    pairs = blkinfo['pairs']

    def stage(e):
        base = (e % RING) * SLOTW
        src = xb[e].rearrange("p c w -> (p c) w")
        nc.sync.dma_start(buf[:, base:base + W], src)
        nc.vector.tensor_copy(buf[:, base + W:base + 2 * W],
                              buf[:, base:base + W])
        nc.vector.tensor_copy(buf[:, base + 2 * W + 1:base + 3 * W],
                              buf[:, base + 1:base + W])
        nc.gpsimd.memset(buf[:, base + 2 * W:base + 2 * W + 1], 0.0)
        if FP8T > 0:
            src8 = xb8[e].rearrange("p c w -> (p c) w")
            nc.gpsimd.dma_start(buf8[:, base:base + W], src8)
            nc.vector.tensor_copy(buf8[:, base + W:base + 2 * W],
                                  buf8[:, base:base + W])
            nc.vector.tensor_copy(buf8[:, base + 2 * W + 1:base + 3 * W],
                                  buf8[:, base + 1:base + W])
            nc.gpsimd.memset(buf8[:, base + 2 * W:base + 2 * W + 1], 0.0)

    staged = min(RING, len(blkinfo['events']))   # hoisted prologue staging
    ltts = [None] * NPAIR
    ltt8s = [None] * NPAIR
    for p in range(min(LTLEAD, NPAIR)):
        ltts[p] = ltts0[0][p]                    # hoisted prologue lt tiles
        ltt8s[p] = ltts0[1][p]

    def voff(em):
        off = em['win'] if not em['zer'] else W + em['win']
        return (em['ev'] % RING) * SLOTW + off

    def emit_pair(p):
        from concourse.ap import AP
        pr = pairs[p]
        ltt = ltts[p]
        ltt8 = ltt8s[p]
        ps = psp.tile([128, W], f32, tag="ps")
        records = pr['records']
        spc = (j == 0 and pairs[p]['hpair'] == 0)
        nmm = len(records) + (NSPEC if spc else 0)
        mi = 0
        started_top = started_bot = False
        for r in records:
            if r['merged']:
                assert not (started_top or started_bot) or \
                    (started_top and started_bot)
                start = not started_top
                started_top = started_bot = True
                out_ap = ps
            elif r['top']:
                start = not started_top
                started_top = True
                out_ap = ps[0:64]
            else:
                start = not started_bot
                started_bot = True
                out_ap = ps[64:128]
            stop = (mi == nmm - 1)
            if r['mode'] == 'f16':
                em = r['parts'][0]
                v = voff(em)
                nc.tensor.matmul(out_ap,
                                 ltt[:, r['ltcol']:r['ltcol'] + r['width']],
                                 buf[:, v:v + W],
                                 start=start, stop=stop)
            else:
                a, bb = r['parts']
                v0, v1 = voff(a), voff(bb)
                assert v0 < v1
                wdt = r['width']
                lhsT = ltt8[:, r['ltcol']:r['ltcol'] + 2 * wdt] \
                    .rearrange("p (t m) -> p t m", t=2)
                base_ap = buf8[:, v0:v0 + W]
                rhs = AP(base_ap.tensor, base_ap.offset,
                         [list(base_ap.ap[0]), [v1 - v0, 2], [1, W]])
                nc.tensor.matmul(out_ap, lhsT, rhs,
                                 start=start, stop=stop,
                                 perf_mode=mybir.MatmulPerfMode.DoubleRow)
            mi += 1
        if spc:
            sbase = (blkinfo['espc'] % RING) * SLOTW
            for jj in range(NSPEC):
                zt = zp.tile([128, W], f16, tag="spz")
                nc.vector.tensor_mul(
                    zt, buf[:, sbase + 255 + jj:sbase + 255 + jj + W],
                    coeft[:, jj * W:(jj + 1) * W])
                nc.tensor.matmul(ps[64:128], ltst[:, jj * O:(jj + 1) * O], zt,
                                 start=False, stop=(mi == nmm - 1))
                mi += 1
        ot = outp.tile([128, W], f32, tag="out")
        nc.scalar.activation(ot, ps,
                             mybir.ActivationFunctionType.Identity,
                             bias=biast, scale=1.0 / LTSCALE)
        # split across issue queues / DMA engines to cut per-pair output
        # latency; extra-split the final pairs (drain-tail critical)
        if p >= NPAIR - 2:
            hw2 = W // 2
            nc.sync.dma_start(outd[p][0:64, 0:hw2], ot[0:64, 0:hw2])
            nc.scalar.dma_start(outd[p][0:64, hw2:W], ot[0:64, hw2:W])
            nc.sync.dma_start(outd[p][64:128, 0:hw2], ot[64:128, 0:hw2])
            nc.scalar.dma_start(outd[p][64:128, hw2:W], ot[64:128, hw2:W])
        else:
            nc.sync.dma_start(outd[p][0:64], ot[0:64])
            nc.scalar.dma_start(outd[p][64:128], ot[64:128])

    for p in range(NPAIR):
        if p >= LTLEAD:
            ltw = max(pairs[p]['ltw'], 64)
            hw2 = ltw // 2
            ltt = ltp.tile([128, aps['LTW']], f16, tag="ltt")
            nc.sync.dma_start(ltt[:, :hw2], lt[p][:, :hw2])
            nc.scalar.dma_start(ltt[:, hw2:ltw], lt[p][:, hw2:ltw])
            ltts[p] = ltt
            if FP8T > 0:
                ltw8 = max(pairs[p]['ltw8'], 64)
                ltt8 = lt8p.tile([128, aps['LT8W']], f8, tag="ltt8")
                nc.sync.dma_start(ltt8[:, :ltw8], lt8[p][:, :ltw8])
                ltt8s[p] = ltt8
            emit_pair(p - LTLEAD)
        while staged < tgt[p]:
            stage(staged)
            staged += 1
    for p in range(NPAIR - LTLEAD, NPAIR):
        emit_pair(p)


tiles_pools = [None]


def _emit_kernel(tc, aps, blocks, E):
    import concourse.mybir as mybir
    nc = tc.nc
    f16 = mybir.dt.float16
    f32 = mybir.dt.float32
    f8 = mybir.dt.float8e4

    with tc.tile_pool(name="bigp", bufs=1) as bigp, \
         tc.tile_pool(name="ltp", bufs=LTLEAD + 2) as ltp, \
         tc.tile_pool(name="lt8p", bufs=LTLEAD + 2) as lt8p, \
         tc.tile_pool(name="zp", bufs=3) as zp, \
         tc.tile_pool(name="psp", bufs=8, space="PSUM") as psp, \
         tc.tile_pool(name="outp", bufs=3) as outp:

        buf = bigp.tile([128, RING * SLOTW], f16)
        buf8 = bigp.tile([128, RING * SLOTW], f8) if FP8T > 0 else None
        coeft = bigp.tile([128, NSPEC * W], f16)
        biast = bigp.tile([128, 1], f32)
        ltst = bigp.tile([128, NSPEC * O], f16)

        blkv = nc.values_load(aps['blkid'][0:1, 0:1],
                              min_val=0, max_val=3,
                              skip_runtime_bounds_check=True)

        # hoisted band-agnostic prologue: lt prefetch for the first LTLEAD
        # pairs interleaved with staging of the first RING events
        # (overlaps the blkid load + dispatch), issue-split across queues
        def stage0(e):
            base = (e % RING) * SLOTW
            src = aps['xb'][e].rearrange("p c w -> (p c) w")
            q = nc.sync if e % 2 == 0 else nc.scalar
            q.dma_start(buf[:, base:base + W], src)
            nc.vector.tensor_copy(buf[:, base + W:base + 2 * W],
                                  buf[:, base:base + W])
            nc.vector.tensor_copy(buf[:, base + 2 * W + 1:base + 3 * W],
                                  buf[:, base + 1:base + W])
            nc.gpsimd.memset(buf[:, base + 2 * W:base + 2 * W + 1], 0.0)
            if FP8T > 0:
                src8 = aps['xb8'][e].rearrange("p c w -> (p c) w")
                nc.gpsimd.dma_start(buf8[:, base:base + W], src8)
                nc.vector.tensor_copy(buf8[:, base + W:base + 2 * W],
                                      buf8[:, base:base + W])
                nc.vector.tensor_copy(
                    buf8[:, base + 2 * W + 1:base + 3 * W],
                    buf8[:, base + 1:base + W])
                nc.gpsimd.memset(buf8[:, base + 2 * W:base + 2 * W + 1],
                                 0.0)

        ltts0 = ([], [])
        for p in range(LTLEAD):
            ltt = ltp.tile([128, aps['LTW']], f16, tag="ltt")
            hw2 = aps['LTW'] // 2
            nc.sync.dma_start(ltt[:, :hw2], aps['lt'][p][:, :hw2])
            nc.scalar.dma_start(ltt[:, hw2:], aps['lt'][p][:, hw2:])
            ltts0[0].append(ltt)
            if FP8T > 0:
                ltt8 = lt8p.tile([128, aps['LT8W']], f8, tag="ltt8")
                nc.sync.dma_start(ltt8, aps['lt8'][p])
                ltts0[1].append(ltt8)
            else:
                ltts0[1].append(None)
            for e in range(4 * p, 4 * p + 4):
                if e < min(RING, E):
                    stage0(e)
        nc.scalar.dma_start(coeft, aps['coefr'])
        nc.scalar.dma_start(biast, aps['biasd'])
        nc.scalar.dma_start(ltst, aps['lts'])
        for e in range(4 * LTLEAD, min(RING, E)):
            stage0(e)

        tiles = (buf, buf8, coeft, biast, ltst)
        tiles_pools[0] = (psp, ltp, lt8p, zp, outp)
        for j in tc.Switch(blkv, 4):
            _emit_pair_section(tc, aps, tiles, blocks[j], j, ltts0)


def _get_compiled():
    """Build tables, schedule, and the Bass program once."""
    if 'prog' in _CACHE:
        return _CACHE['prog']
    import concourse.mybir as mybir
    import concourse.tile as tile
    from concourse import bacc

    tt = _build_tap_tables()
    blocks, E, (LTW, LT8W) = _build_schedule(tt)
    # special-slot event index (input row of tap (1,1)) for band 0
    espc = None
    iy_spc = int(np.clip(tt['iy0'][1, 1], 0, 255))
    for ei, iy in enumerate(blocks[0]['events']):
        if iy == iy_spc:
            espc = ei
            break
    assert espc is not None
    blocks[0]['espc'] = espc
    for j in range(1, 4):
        blocks[j]['espc'] = 0

    f16 = mybir.dt.float16
    f32 = mybir.dt.float32
    nc = bacc.Bacc("TRN2", target_bir_lowering=False, debug=False,
                   num_devices=NCORES)
    f8 = mybir.dt.float8e4
    aps = {
        'xb': nc.dram_tensor("xb", [E, 2, C, W], f16,
                             kind="ExternalInput").ap(),
        'xb8': nc.dram_tensor("xb8", [E, 2, C, W], f8,
                              kind="ExternalInput").ap(),
        'lt': nc.dram_tensor("lt", [NPAIR, 128, LTW], f16,
                             kind="ExternalInput").ap(),
        'lt8': nc.dram_tensor("lt8", [NPAIR, 128, LT8W], f8,
                              kind="ExternalInput").ap(),
        'lts': nc.dram_tensor("lts", [128, NSPEC * O], f16,
                              kind="ExternalInput").ap(),
        'blkid': nc.dram_tensor("blkid", [1, 1], mybir.dt.int32,
                                kind="ExternalInput").ap(),
        'coefr': nc.dram_tensor("coefr", [128, NSPEC * W], f16,
                                kind="ExternalInput").ap(),
        'biasd': nc.dram_tensor("biasd", [128, 1], f32,
                                kind="ExternalInput").ap(),
        'out': nc.dram_tensor("out", [NPAIR, 128, W], f32,
                              kind="ExternalOutput").ap(),
        'LTW': LTW,
        'LT8W': LT8W,
    }
    with tile.TileContext(nc) as tc:
        _emit_kernel(tc, aps, blocks, E)
    nc.finalize()

    _CACHE['prog'] = (nc, tt, blocks, E, (LTW, LT8W))
    return _CACHE['prog']


def _core_inputs(x, weight, bias, tt, blocks, E, LTWS):
    """Assemble per-core in_maps. Core c = batch (c // 4), band (c % 4)."""
    import ml_dtypes
    f8d = ml_dtypes.float8_e4m3
    LTW, LT8W = LTWS
    w3 = weight.reshape(O, C, K).astype(np.float64)
    # wT[k]: [c, o] weight slice per tap
    wT = [np.ascontiguousarray(w3[:, :, k].T) for k in range(K)]
    biasd = np.ascontiguousarray(
        np.concatenate([bias, bias]).reshape(128, 1).astype(np.float32))

    lts_on = np.zeros((128, NSPEC * O), np.float16)
    for jj in range(NSPEC):
        lts_on[:C, jj * O:(jj + 1) * O] = \
            (LTSCALE * wT[1]).astype(np.float16)
    lts_off = np.zeros((128, NSPEC * O), np.float16)

    Gam = tt['Gam'].astype(np.float16)
    coef_on = np.ascontiguousarray(
        np.broadcast_to(Gam[:, None, :], (NSPEC, 128, W))
        .transpose(1, 0, 2).reshape(128, NSPEC * W))
    coef_off = np.zeros((128, NSPEC * W), np.float16)

    def fill_em(dst, c, em, width):
        cb = c + 64 if em['merged'] else c
        for k, (sc0, sc1) in em['top'].items():
            dst[0:64, c:c + 64] += LTSCALE * sc0 * wT[k]
            dst[64:128, c:c + 64] += LTSCALE * sc1 * wT[k]
        for k, (sc0, sc1) in em['bot'].items():
            dst[0:64, cb:cb + 64] += LTSCALE * sc0 * wT[k]
            dst[64:128, cb:cb + 64] += LTSCALE * sc1 * wT[k]

    lt_blk, lt8_blk = [], []
    for blk in range(4):
        ltv = np.zeros((NPAIR, 128, LTW), np.float64)
        ltv8 = np.zeros((NPAIR, 128, LT8W), np.float64)
        for p in range(NPAIR):
            for r in blocks[blk]['pairs'][p]['records']:
                if r['mode'] == 'f16':
                    fill_em(ltv[p], r['ltcol'], r['parts'][0], r['width'])
                else:
                    a, bb = r['parts']
                    fill_em(ltv8[p], r['ltcol'], a, r['width'])
                    fill_em(ltv8[p], r['ltcol'] + r['width'], bb,
                            r['width'])
        lt_blk.append(ltv.astype(np.float16))
        lt8_blk.append(ltv8.astype(np.float16).astype(f8d))

    in_maps = []
    for cid in range(NCORES):
        b, blk = cid // 4, cid % 4
        xz = np.concatenate([x[b], np.zeros((C, 1, W), x.dtype)], axis=1)
        xz = xz.astype(np.float16)
        rows = np.asarray(blocks[blk]['events'], np.int64)
        pair_idx = np.stack([rows, rows + 1], axis=1)       # [E, 2]
        xbv = xz[:, pair_idx, :]                            # [C, E, 2, W]
        xbv = np.ascontiguousarray(xbv.transpose(1, 2, 0, 3))  # [E,2,C,W]
        in_maps.append({
            'xb': xbv,
            'xb8': np.ascontiguousarray(xbv.astype(f8d)),
            'lt': lt_blk[blk],
            'lt8': lt8_blk[blk],
            'lts': lts_on if blk == 0 else lts_off,
            'blkid': np.array([[blk]], np.int32),
            'coefr': coef_on if blk == 0 else coef_off,
            'biasd': biasd,
        })
    return in_maps


def _gather(res):
    """Assemble full output from per-core results (iteration order -> rows)."""
    blocks = _CACHE['prog'][2]
    out = np.empty((B, O, H, W), np.float32)
    for cid in range(NCORES):
        b, blk = cid // 4, cid % 4
        oc = res.results[cid]['out']                        # [NPAIR, 128, W]
        hps = np.array([pr['hpair']
                        for pr in blocks[blk]['pairs']], np.int64)
        rows0 = blk * NROW + 2 * hps
        out[b, :, rows0, :] = oc[:, 0:64, :]
        out[b, :, rows0 + 1, :] = oc[:, 64:128, :]
    return out


def kernel(x, weight, bias):
    from concourse.bass_utils import run_bass_kernel_spmd
    x = np.asarray(x, dtype=np.float32)
    weight = np.asarray(weight, dtype=np.float32)
    bias = np.asarray(bias, dtype=np.float32)

    nc, tt, blocks, E, LTW = _get_compiled()
    in_maps = _core_inputs(x, weight, bias, tt, blocks, E, LTW)
    res = run_bass_kernel_spmd(nc, in_maps, core_ids=list(range(NCORES)))
    return _gather(res)


# revision 51
# speedup vs baseline: 1.0320x; 1.0320x over previous
"""Trainium2 Bass kernel for nn_EquiConv2d (equirectangular deformable conv).

Key structural facts exploited (derived from the reference geometry):
  * off_y is exactly longitude-invariant, so each (tap k, row h) samples two
    fixed input rows (iy0, iy0+1) with a constant y-fraction.
  * off_x is longitude-invariant up to the 2*pi wrap: sampling along a row is
    a CIRCULAR shift by a constant s0(k,h) plus a constant x-fraction.
  * Hence the deformable conv is a set of matmul "slots" per output row
    ([128=(c x row-pair) contraction, 512 free]) reading circularly
    duplicated row-pair tiles at per-(k,h) column offsets, with the bilinear
    corner weights folded into the stationary (weight) operand.
  * NEW vs baseline: output rows are processed in PAIRS sharing one PSUM
    bank (top 64 partitions = even row, bottom 64 = odd row).  Slots of the
    two rows that read the SAME moving stream (same input row-pair event,
    same column window, same wrap-seam variant) are MERGED into a single
    matmul with a [128, 128] stationary — the ky-ladder of the equirect
    geometry makes ~30% of all slots mergeable, cutting tensor-engine work
    by the same fraction with bit-identical arithmetic.
  * Two fp32 oddities handled exactly: tap (k=7,h=255) is identically zero
    and tap (k=1,h=1) samples near the antipode with fp32-noise-scattered
    positions -> handled by 3 extra matmul slots with per-column coefficient
    vectors (data-driven, active only on the cores owning global row 1).

Sharding: 8 cores = 2 batches x 4 bands of 32 output-row pairs.
"""

import math

import numpy as np

# ----------------------------------------------------------------------------
# problem constants
B, C, H, W = 2, 64, 256, 512
O, KH, KW = 64, 3, 3
K = KH * KW
NCORES = 8
NROW = 64            # output rows per core
NPAIR = NROW // 2    # output row-pairs per core
NSPEC = 3            # special (antipode) slots, accumulated into local row 1
RING = 32            # staged row-pair ring slots
PF = 2               # staging prefetch lead (row-pairs)
LTLEAD = 3           # lt-table DMA prefetch lead (row-pairs)
SLOTW = 1536         # [A=row | B=row | Z=row w/ col0 zeroed] per ring slot
SKIP_TOL = 1e-4      # drop matmul slots with |scale| below this
FP8T = 0.0           # max |stationary scale| for fp8 DoubleRow slots (0=off)
LTSCALE = 64.0       # stationary pre-scale (fp8 normal range); undone in
                     # the output activation via scale=1/LTSCALE

_CACHE = {}


# ----------------------------------------------------------------------------
# host-side geometry tables (must replicate reference fp32 semantics exactly)

def _compute_offsets_jax():
    """Bit-exact replica of reference.equi_offsets on jax CPU."""
    import jax
    import jax.numpy as jnp
    cpu = jax.devices("cpu")[0]
    with jax.default_device(cpu):
        dtype = jnp.float32
        pano_H, pano_W, kH, kW = H, W, KH, KW
        Kk = kH * kW
        u = jnp.arange(pano_W, dtype=dtype)
        v = jnp.arange(pano_H, dtype=dtype)
        phi = (u - pano_W / 2.0) / pano_W * (2.0 * math.pi)
        theta = -(v - pano_H / 2.0) / pano_H * math.pi
        cp, sp = jnp.cos(phi), jnp.sin(phi)
        z, one = jnp.zeros_like(cp), jnp.ones_like(cp)
        Ry = jnp.stack([jnp.stack([cp, z, sp], -1),
                        jnp.stack([z, one, z], -1),
                        jnp.stack([-sp, z, cp], -1)], -2)
        ct, st = jnp.cos(theta), jnp.sin(theta)
        zh, oh = jnp.zeros_like(ct), jnp.ones_like(ct)
        Rx = jnp.stack([jnp.stack([oh, zh, zh], -1),
                        jnp.stack([zh, ct, -st], -1),
                        jnp.stack([zh, st, ct], -1)], -2)
        ROT = jnp.einsum('wij,hjk->hwik', Ry, Rx)
        fov_w = kW * (2.0 * math.pi / pano_W)
        focal = (kW / 2.0) / math.tan(fov_w / 2.0)
        hg = (jnp.arange(kH, dtype=dtype)[:, None] + 0.5 - kH / 2.0)
        wg = (jnp.arange(kW, dtype=dtype)[None, :] + 0.5 - kW / 2.0)
        hg = jnp.broadcast_to(hg, (kH, kW)).reshape(Kk)
        wg = jnp.broadcast_to(wg, (kH, kW)).reshape(Kk)
        rays0 = jnp.stack([wg / focal, hg / focal, jnp.ones(Kk, dtype)], 0)
        rays0 = rays0 / jnp.linalg.norm(rays0, axis=0, keepdims=True)
        rays = jnp.einsum('hwik,kn->hwin', ROT, rays0)
        phi2 = jnp.arctan2(rays[..., 0, :], rays[..., 2, :])
        th2 = jnp.arcsin(jnp.clip(rays[..., 1, :], -1.0, 1.0))
        x = pano_W / (2.0 * math.pi) * phi2 + pano_W / 2.0
        y = pano_H / math.pi * th2 + pano_H / 2.0
        off_x = x - (wg[None, None, :] + u[None, :, None])
        off_y = y - (hg[None, None, :] + v[:, None, None])
        return (np.asarray(jnp.transpose(off_y, (2, 0, 1))),
                np.asarray(jnp.transpose(off_x, (2, 0, 1))))


def _build_tap_tables():
    off_y, off_x = _compute_offsets_jax()
    ky = np.repeat(np.arange(KH), KW).astype(np.float32)
    kx = np.tile(np.arange(KW), KH).astype(np.float32)
    base_x = (np.arange(W, dtype=np.float32) - np.float32(1))
    base_y = (np.arange(H, dtype=np.float32) - np.float32(1))
    px = (base_x[None, None, :] + kx[:, None, None] + off_x).astype(np.float32)
    py = (base_y[None, :, None] + ky[:, None, None] + off_y).astype(np.float32)
    pyc = py[:, :, 0]
    assert np.all(py == pyc[:, :, None]), "off_y not longitude-invariant"

    iy0 = np.floor(pyc).astype(np.int64)
    wy1 = (pyc - np.floor(pyc)).astype(np.float64)
    v0 = (iy0 >= 0) & (iy0 < H)
    v1 = (iy0 + 1 >= 0) & (iy0 + 1 < H)
    cy0 = np.where(v0, 1.0 - wy1, 0.0)
    cy1 = np.where(v1, wy1, 0.0)

    Draw = np.mod((px.astype(np.float64) - np.arange(W)[None, None, :]), 512.0)
    ang = Draw / 512.0 * 2 * np.pi
    mean = np.mod(np.angle(np.exp(1j * ang).mean(axis=2)) / (2 * np.pi) * 512.0,
                  512.0)
    resid = np.mod(Draw - mean[:, :, None] + 256.0, 512.0) - 256.0
    D = mean + np.median(resid, axis=2)
    s0 = np.mod(np.floor(D), 512).astype(np.int64)
    frac = D - np.floor(D)

    special = np.zeros((K, H), dtype=bool)
    special[1, 1] = True
    dead = (cy0 == 0.0) & (cy1 == 0.0)

    Ddev = np.abs(np.mod(Draw - D[:, :, None] + 256.0, 512.0) - 256.0)
    dev = Ddev.max(axis=2)
    bad = (dev > 5e-4) & ~special & ~dead
    assert not bad.any(), f"unrepresentable taps: {np.argwhere(bad)}"

    def ref_coefs(p):
        x0 = math.floor(p)
        fr = p - x0
        out = {}
        for ix, wt in ((x0, 1.0 - fr), (x0 + 1, fr)):
            if 0 <= ix < W and wt != 0.0:
                out[ix] = out.get(ix, 0.0) + wt
        return out

    # seam variant selection: decided by the exact fp32 px at the wrap column
    slot0_useG = np.zeros((K, H), dtype=bool)
    slot1_useF = np.zeros((K, H), dtype=bool)
    for k in range(K):
        for h in range(H):
            if special[k, h] or dead[k, h]:
                continue
            s = int(s0[k, h]); fr = frac[k, h]
            if s >= 1:
                w0 = (512 - s) % 512
                rc = ref_coefs(float(px[k, h, w0]))
                slot0_useG[k, h] = (abs(rc.get(0, 0.0))
                                    < abs(rc.get(0, 0.0) - (1 - fr)))
            w1 = (511 - s) % 512
            rc = ref_coefs(float(px[k, h, w1]))
            slot1_useF[k, h] = (abs(rc.get(0, 0.0) - fr)
                                < abs(rc.get(0, 0.0)))

    # special tap (1,1): per-column coefficients on F offsets 255..257
    pxs = px[1, 1, :].astype(np.float64)
    Gam = np.zeros((3, W), dtype=np.float64)
    for w in range(W):
        p = pxs[w]
        x0 = math.floor(p)
        fr = p - x0
        for ix, wt in ((x0, 1.0 - fr), (x0 + 1, fr)):
            if 0 <= ix < W and wt != 0.0:
                found = False
                for jj in range(3):
                    if (255 + jj + w) % 512 == ix % 512:
                        Gam[jj, w] += wt
                        found = True
                        break
                assert found, (w, p, ix)

    return dict(iy0=iy0, cy0=cy0, cy1=cy1, s0=s0, frac=frac,
                slot0_useG=slot0_useG, slot1_useF=slot1_useF,
                special=special, dead=dead, Gam=Gam)


# ----------------------------------------------------------------------------
# slot -> (event row, window, variant) keys + merged emit schedule

def _row_slots(tt, h):
    """Slots of output row h: list of (key, k, half_scales).
    key = (input_row, window_col, zeroed_variant);
    scales = (coef_row0, coef_row1) to fold into the stationary halves."""
    out = []
    for k in range(K):
        if tt['dead'][k, h] or tt['special'][k, h]:
            continue
        s = int(tt['s0'][k, h]); fr = float(tt['frac'][k, h])
        iy = int(np.clip(tt['iy0'][k, h], 0, 255))
        c0 = float(tt['cy0'][k, h]); c1 = float(tt['cy1'][k, h])
        cmax = max(abs(c0), abs(c1))
        if (1.0 - fr) * cmax >= SKIP_TOL:
            zer = bool(tt['slot0_useG'][k, h]) and s >= 1
            out.append(((iy, s, zer), k, (c0 * (1 - fr), c1 * (1 - fr))))
        if fr * cmax >= SKIP_TOL:
            zer = not bool(tt['slot1_useF'][k, h])
            out.append(((iy, s + 1, zer), k, (c0 * fr, c1 * fr)))
    return out


def _build_schedule(tt):
    """Per band: event list (input row-pairs, in first-use order over pairs),
    staging targets, and per-pair emit lists.

    emit = dict(ev, win, zer, top=[(k,c0,c1)...], bot=[...], ltcol, width)
    ordered merged-first (within groups by event index)."""
    iy_spc = int(np.clip(tt['iy0'][1, 1], 0, 255))
    blocks = []
    for blk in range(4):
        ev_of, events, first_use = {}, [], []
        pairs = []
        # process polar-heavy pairs (many distinct input rows) LAST so the
        # staging ring warms up on cheap pairs instead of stalling the PE
        order = list(range(NPAIR))
        for it, hp in enumerate(order):
            h0 = blk * NROW + 2 * hp
            h1 = h0 + 1
            if blk == 0 and hp == 0 and iy_spc not in ev_of:
                ev_of[iy_spc] = len(events)
                events.append(iy_spc)
                first_use.append(it)
            keymap = {}
            for (key, k, sc) in _row_slots(tt, h0):
                keymap.setdefault(key, (dict(), dict()))[0].setdefault(
                    k, [0.0, 0.0])
                e = keymap[key][0][k]
                e[0] += sc[0]; e[1] += sc[1]
            for (key, k, sc) in _row_slots(tt, h1):
                keymap.setdefault(key, (dict(), dict()))[1].setdefault(
                    k, [0.0, 0.0])
                e = keymap[key][1][k]
                e[0] += sc[0]; e[1] += sc[1]
            # register events (input rows) in deterministic order
            emits = []
            for key in keymap:
                iy = key[0]
                if iy not in ev_of:
                    ev_of[iy] = len(events)
                    events.append(iy)
                    first_use.append(it)
                top, bot = keymap[key]
                emits.append(dict(ev=ev_of[iy], win=key[1], zer=key[2],
                                  top=top, bot=bot,
                                  merged=bool(top) and bool(bot)))
            # merged first, then solos; within each group by event order
            emits.sort(key=lambda em: (not em['merged'], em['ev'], em['win']))
            for em in emits:
                em['width'] = 128 if em['merged'] else 64
                em['mode'] = 'f16'

            # fp8 DoubleRow pairing: fuse two low-|scale| emits with the
            # same psum region into one half-rate fp8 matmul
            def maxsc(em):
                m = 0.0
                for dd in (em['top'], em['bot']):
                    for _k, (s0, s1) in dd.items():
                        m = max(m, abs(s0), abs(s1))
                return m

            def voff(em):
                off = em['win'] if not em['zer'] else W + em['win']
                return (em['ev'] % RING) * SLOTW + off

            records = []
            if FP8T > 0:
                groups = {}
                for em in emits:
                    if em['merged']:
                        continue   # DoubleRow caps stationary at 2x64 cols
                    reg = 't' if em['top'] else 'b'
                    if maxsc(em) <= FP8T:
                        groups.setdefault(reg, []).append(em)
                for reg, ems in groups.items():
                    ems.sort(key=voff)
                    i = 0
                    while i + 1 < len(ems):
                        a, bb = ems[i], ems[i + 1]
                        if a['width'] == bb['width']:
                            a['mode'] = bb['mode'] = 'f8'
                            records.append(dict(
                                mode='f8', merged=a['merged'],
                                top=a['top'] or bb['top'],
                                bot=a['bot'] or bb['bot'],
                                width=a['width'], parts=(a, bb),
                                ev=min(a['ev'], bb['ev'])))
                            i += 2
                        else:
                            i += 1
            for em in emits:
                if em['mode'] == 'f16':
                    records.append(dict(mode='f16', merged=em['merged'],
                                        top=em['top'], bot=em['bot'],
                                        width=em['width'], parts=(em,),
                                        ev=em['ev']))
            # psum-start safety: region-covering (merged) records first
            records.sort(key=lambda r: (not r['merged'], r['ev']))

            # assign lt columns (f16) and lt8 columns (fp8, 2 blocks each)
            col = col8 = 0
            for r in records:
                if r['mode'] == 'f16':
                    r['ltcol'] = col
                    col += r['width']
                else:
                    r['ltcol'] = col8
                    col8 += 2 * r['width']
            pairs.append(dict(emits=emits, records=records,
                              ltw=col, ltw8=col8, hpair=hp))
        blocks.append(dict(events=events, first_use=first_use, pairs=pairs))

    E = max(len(b['events']) for b in blocks)
    LTW = max(pr['ltw'] for b in blocks for pr in b['pairs'])
    LT8W = max(max(pr['ltw8'] for b in blocks for pr in b['pairs']), 64)
    for b in blocks:
        while len(b['events']) < E:
            b['events'].append(b['events'][-1])

    # staging target per pair: staged-count needed before pair p is
    # tgt[p] = U[min(p+PF, NPAIR-1)] where U = events first-used by <= p
    for b in blocks:
        fu = np.asarray(b['first_use'])
        Uv = np.array([int(np.searchsorted(fu, p, 'right'))
                       for p in range(NPAIR)])
        b['tgt'] = [int(Uv[min(p + PF, NPAIR - 1)]) for p in range(NPAIR)]
        # ring-overwrite feasibility: stage(e) is issued in iteration ls[e]
        # AFTER emitting pair ls[e]-LTLEAD, so every reader of the slot it
        # overwrites (event e-RING) must have been emitted by then.  Matmul
        # emission lags staging by LTLEAD pairs, hence the margin.
        ls = np.full(E, NPAIR, np.int64)
        tgt = np.asarray(b['tgt'])
        for e in range(E):
            hit = np.where(tgt > e)[0]
            if len(hit):
                ls[e] = hit[0]
        lastuse = {}
        for p in range(NPAIR):
            for em in b['pairs'][p]['emits']:
                lastuse[em['ev']] = p
        for e in range(RING, E):
            if e - RING in lastuse:
                assert lastuse[e - RING] <= ls[e] - LTLEAD, \
                    f"RING={RING} too small: ev{e} overwrites ev{e-RING} " \
                    f"(lastuse pair {lastuse[e-RING]}, staged in it " \
                    f"{ls[e]}, emit lag {LTLEAD})"
    return blocks, E, (LTW, LT8W)


# ----------------------------------------------------------------------------
# device program

def _emit_pair_section(tc, aps, tiles, blkinfo, j, ltts0):
    """Emit one per-band section (all-static APs)."""
    import concourse.mybir as mybir
    nc = tc.nc
    f16 = mybir.dt.float16
    f32 = mybir.dt.float32
    f8 = mybir.dt.float8e4
    buf, buf8, coeft, biast, ltst = tiles
    xb, outd, lt = aps['xb'], aps['out'], aps['lt']
    xb8, lt8 = aps['xb8'], aps['lt8']
    psp, ltp, lt8p, zp, outp = tiles_pools[0]
    tgt = blkinfo['tgt']
    pairs = blkinfo['pairs']

    def stage(e):
        base = (e % RING) * SLOTW
        src = xb[e].rearrange("p c w -> (p c) w")
        nc.sync.dma_start(buf[:, base:base + W], src)
        nc.vector.tensor_copy(buf[:, base + W:base + 2 * W],
                              buf[:, base:base + W])
        nc.vector.tensor_copy(buf[:, base + 2 * W + 1:base + 3 * W],
                              buf[:, base + 1:base + W])
        nc.gpsimd.memset(buf[:, base + 2 * W:base + 2 * W + 1], 0.0)
        if FP8T > 0:
            src8 = xb8[e].rearrange("p c w -> (p c) w")
            nc.gpsimd.dma_start(buf8[:, base:base + W], src8)
            nc.vector.tensor_copy(buf8[:, base + W:base + 2 * W],
                                  buf8[:, base:base + W])
            nc.vector.tensor_copy(buf8[:, base + 2 * W + 1:base + 3 * W],
                                  buf8[:, base + 1:base + W])
            nc.gpsimd.memset(buf8[:, base + 2 * W:base + 2 * W + 1], 0.0)

    staged = min(RING, len(blkinfo['events']))   # hoisted prologue staging
    ltts = [None] * NPAIR
    ltt8s = [None] * NPAIR
    for p in range(min(LTLEAD, NPAIR)):
        ltts[p] = ltts0[0][p]                    # hoisted prologue lt tiles
        ltt8s[p] = ltts0[1][p]

    def voff(em):
        off = em['win'] if not em['zer'] else W + em['win']
        return (em['ev'] % RING) * SLOTW + off

    def emit_pair(p):
        from concourse.ap import AP
        pr = pairs[p]
        ltt = ltts[p]
        ltt8 = ltt8s[p]
        ps = psp.tile([128, W], f32, tag="ps")
        records = pr['records']
        spc = (j == 0 and pairs[p]['hpair'] == 0)
        nmm = len(records) + (NSPEC if spc else 0)
        mi = 0
        started_top = started_bot = False
        for r in records:
            if r['merged']:
                assert not (started_top or started_bot) or \
                    (started_top and started_bot)
                start = not started_top
                started_top = started_bot = True
                out_ap = ps
            elif r['top']:
                start = not started_top
                started_top = True
                out_ap = ps[0:64]
            else:
                start = not started_bot
                started_bot = True
                out_ap = ps[64:128]
            stop = (mi == nmm - 1)
            if r['mode'] == 'f16':
                em = r['parts'][0]
                v = voff(em)
                nc.tensor.matmul(out_ap,
                                 ltt[:, r['ltcol']:r['ltcol'] + r['width']],
                                 buf[:, v:v + W],
                                 start=start, stop=stop)
            else:
                a, bb = r['parts']
                v0, v1 = voff(a), voff(bb)
                assert v0 < v1
                wdt = r['width']
                lhsT = ltt8[:, r['ltcol']:r['ltcol'] + 2 * wdt] \
                    .rearrange("p (t m) -> p t m", t=2)
                base_ap = buf8[:, v0:v0 + W]
                rhs = AP(base_ap.tensor, base_ap.offset,
                         [list(base_ap.ap[0]), [v1 - v0, 2], [1, W]])
                nc.tensor.matmul(out_ap, lhsT, rhs,
                                 start=start, stop=stop,
                                 perf_mode=mybir.MatmulPerfMode.DoubleRow)
            mi += 1
        if spc:
            sbase = (blkinfo['espc'] % RING) * SLOTW
            for jj in range(NSPEC):
                zt = zp.tile([128, W], f16, tag="spz")
                nc.vector.tensor_mul(
                    zt, buf[:, sbase + 255 + jj:sbase + 255 + jj + W],
                    coeft[:, jj * W:(jj + 1) * W])
                nc.tensor.matmul(ps[64:128], ltst[:, jj * O:(jj + 1) * O], zt,
                                 start=False, stop=(mi == nmm - 1))
                mi += 1
        ot = outp.tile([128, W], f32, tag="out")
        nc.scalar.activation(ot, ps,
                             mybir.ActivationFunctionType.Identity,
                             bias=biast, scale=1.0 / LTSCALE)
        # split across issue queues / DMA engines to cut per-pair output
        # latency; extra-split the final pairs (drain-tail critical)
        if p >= NPAIR - 2:
            hw2 = W // 2
            nc.sync.dma_start(outd[p][0:64, 0:hw2], ot[0:64, 0:hw2])
            nc.scalar.dma_start(outd[p][0:64, hw2:W], ot[0:64, hw2:W])
            nc.sync.dma_start(outd[p][64:128, 0:hw2], ot[64:128, 0:hw2])
            nc.scalar.dma_start(outd[p][64:128, hw2:W], ot[64:128, hw2:W])
        else:
            nc.sync.dma_start(outd[p][0:64], ot[0:64])
            nc.scalar.dma_start(outd[p][64:128], ot[64:128])

    for p in range(NPAIR):
        if p >= LTLEAD:
            ltw = max(pairs[p]['ltw'], 64)
            hw2 = ltw // 2
            ltt = ltp.tile([128, aps['LTW']], f16, tag="ltt")
            nc.sync.dma_start(ltt[:, :hw2], lt[p][:, :hw2])
            nc.scalar.dma_start(ltt[:, hw2:ltw], lt[p][:, hw2:ltw])
            ltts[p] = ltt
            if FP8T > 0:
                ltw8 = max(pairs[p]['ltw8'], 64)
                ltt8 = lt8p.tile([128, aps['LT8W']], f8, tag="ltt8")
                nc.sync.dma_start(ltt8[:, :ltw8], lt8[p][:, :ltw8])
                ltt8s[p] = ltt8
            emit_pair(p - LTLEAD)
        while staged < tgt[p]:
            stage(staged)
            staged += 1
    for p in range(NPAIR - LTLEAD, NPAIR):
        emit_pair(p)


tiles_pools = [None]


def _emit_kernel(tc, aps, blocks, E):
    import concourse.mybir as mybir
    nc = tc.nc
    f16 = mybir.dt.float16
    f32 = mybir.dt.float32
    f8 = mybir.dt.float8e4

    with tc.tile_pool(name="bigp", bufs=1) as bigp, \
         tc.tile_pool(name="ltp", bufs=LTLEAD + 2) as ltp, \
         tc.tile_pool(name="lt8p", bufs=LTLEAD + 2) as lt8p, \
         tc.tile_pool(name="zp", bufs=3) as zp, \
         tc.tile_pool(name="psp", bufs=8, space="PSUM") as psp, \
         tc.tile_pool(name="outp", bufs=3) as outp:

        buf = bigp.tile([128, RING * SLOTW], f16)
        buf8 = bigp.tile([128, RING * SLOTW], f8) if FP8T > 0 else None
        coeft = bigp.tile([128, NSPEC * W], f16)
        biast = bigp.tile([128, 1], f32)
        ltst = bigp.tile([128, NSPEC * O], f16)

        blkv = nc.values_load(aps['blkid'][0:1, 0:1],
                              min_val=0, max_val=3,
                              skip_runtime_bounds_check=True)

        # hoisted band-agnostic prologue: lt prefetch for the first LTLEAD
        # pairs interleaved with staging of the first RING events
        # (overlaps the blkid load + dispatch), issue-split across queues
        def stage0(e):
            base = (e % RING) * SLOTW
            src = aps['xb'][e].rearrange("p c w -> (p c) w")
            q = nc.sync if e % 2 == 0 else nc.scalar
            q.dma_start(buf[:, base:base + W], src)
            nc.vector.tensor_copy(buf[:, base + W:base + 2 * W],
                                  buf[:, base:base + W])
            nc.vector.tensor_copy(buf[:, base + 2 * W + 1:base + 3 * W],
                                  buf[:, base + 1:base + W])
            nc.gpsimd.memset(buf[:, base + 2 * W:base + 2 * W + 1], 0.0)
            if FP8T > 0:
                src8 = aps['xb8'][e].rearrange("p c w -> (p c) w")
                nc.gpsimd.dma_start(buf8[:, base:base + W], src8)
                nc.vector.tensor_copy(buf8[:, base + W:base + 2 * W],
                                      buf8[:, base:base + W])
                nc.vector.tensor_copy(
                    buf8[:, base + 2 * W + 1:base + 3 * W],
                    buf8[:, base + 1:base + W])
                nc.gpsimd.memset(buf8[:, base + 2 * W:base + 2 * W + 1],
                                 0.0)

        ltts0 = ([], [])
        for p in range(LTLEAD):
            ltt = ltp.tile([128, aps['LTW']], f16, tag="ltt")
            hw2 = aps['LTW'] // 2
            nc.sync.dma_start(ltt[:, :hw2], aps['lt'][p][:, :hw2])
            nc.scalar.dma_start(ltt[:, hw2:], aps['lt'][p][:, hw2:])
            ltts0[0].append(ltt)
            if FP8T > 0:
                ltt8 = lt8p.tile([128, aps['LT8W']], f8, tag="ltt8")
                nc.sync.dma_start(ltt8, aps['lt8'][p])
                ltts0[1].append(ltt8)
            else:
                ltts0[1].append(None)
            for e in range(4 * p, 4 * p + 4):
                if e < min(RING, E):
                    stage0(e)
        nc.scalar.dma_start(coeft, aps['coefr'])
        nc.scalar.dma_start(biast, aps['biasd'])
        nc.scalar.dma_start(ltst, aps['lts'])
        for e in range(4 * LTLEAD, min(RING, E)):
            stage0(e)

        tiles = (buf, buf8, coeft, biast, ltst)
        tiles_pools[0] = (psp, ltp, lt8p, zp, outp)
        for j in tc.Switch(blkv, 4):
            _emit_pair_section(tc, aps, tiles, blocks[j], j, ltts0)


def _get_compiled():
    """Build tables, schedule, and the Bass program once."""
    if 'prog' in _CACHE:
        return _CACHE['prog']
    import concourse.mybir as mybir
    import concourse.tile as tile
    from concourse import bacc

    tt = _build_tap_tables()
    blocks, E, (LTW, LT8W) = _build_schedule(tt)
    # special-slot event index (input row of tap (1,1)) for band 0
    espc = None
    iy_spc = int(np.clip(tt['iy0'][1, 1], 0, 255))
    for ei, iy in enumerate(blocks[0]['events']):
        if iy == iy_spc:
            espc = ei
            break
    assert espc is not None
    blocks[0]['espc'] = espc
    for j in range(1, 4):
        blocks[j]['espc'] = 0

    f16 = mybir.dt.float16
    f32 = mybir.dt.float32
    nc = bacc.Bacc("TRN2", target_bir_lowering=False, debug=False,
                   num_devices=NCORES)
    f8 = mybir.dt.float8e4
    aps = {
        'xb': nc.dram_tensor("xb", [E, 2, C, W], f16,
                             kind="ExternalInput").ap(),
        'xb8': nc.dram_tensor("xb8", [E, 2, C, W], f8,
                              kind="ExternalInput").ap(),
        'lt': nc.dram_tensor("lt", [NPAIR, 128, LTW], f16,
                             kind="ExternalInput").ap(),
        'lt8': nc.dram_tensor("lt8", [NPAIR, 128, LT8W], f8,
                              kind="ExternalInput").ap(),
        'lts': nc.dram_tensor("lts", [128, NSPEC * O], f16,
                              kind="ExternalInput").ap(),
        'blkid': nc.dram_tensor("blkid", [1, 1], mybir.dt.int32,
                                kind="ExternalInput").ap(),
        'coefr': nc.dram_tensor("coefr", [128, NSPEC * W], f16,
                                kind="ExternalInput").ap(),
        'biasd': nc.dram_tensor("biasd", [128, 1], f32,
                                kind="ExternalInput").ap(),
        'out': nc.dram_tensor("out", [NPAIR, 128, W], f32,
                              kind="ExternalOutput").ap(),
        'LTW': LTW,
        'LT8W': LT8W,
    }
    with tile.TileContext(nc) as tc:
        _emit_kernel(tc, aps, blocks, E)
    nc.finalize()

    _CACHE['prog'] = (nc, tt, blocks, E, (LTW, LT8W))
    return _CACHE['prog']


def _core_inputs(x, weight, bias, tt, blocks, E, LTWS):
    """Assemble per-core in_maps. Core c = batch (c // 4), band (c % 4)."""
    import ml_dtypes
    f8d = ml_dtypes.float8_e4m3
    LTW, LT8W = LTWS
    w3 = weight.reshape(O, C, K).astype(np.float64)
    # wT[k]: [c, o] weight slice per tap
    wT = [np.ascontiguousarray(w3[:, :, k].T) for k in range(K)]
    biasd = np.ascontiguousarray(
        np.concatenate([bias, bias]).reshape(128, 1).astype(np.float32))

    lts_on = np.zeros((128, NSPEC * O), np.float16)
    for jj in range(NSPEC):
        lts_on[:C, jj * O:(jj + 1) * O] = \
            (LTSCALE * wT[1]).astype(np.float16)
    lts_off = np.zeros((128, NSPEC * O), np.float16)

    Gam = tt['Gam'].astype(np.float16)
    coef_on = np.ascontiguousarray(
        np.broadcast_to(Gam[:, None, :], (NSPEC, 128, W))
        .transpose(1, 0, 2).reshape(128, NSPEC * W))
    coef_off = np.zeros((128, NSPEC * W), np.float16)

    def fill_em(dst, c, em, width):
        cb = c + 64 if em['merged'] else c
        for k, (sc0, sc1) in em['top'].items():
            dst[0:64, c:c + 64] += LTSCALE * sc0 * wT[k]
            dst[64:128, c:c + 64] += LTSCALE * sc1 * wT[k]
        for k, (sc0, sc1) in em['bot'].items():
            dst[0:64, cb:cb + 64] += LTSCALE * sc0 * wT[k]
            dst[64:128, cb:cb + 64] += LTSCALE * sc1 * wT[k]

    lt_blk, lt8_blk = [], []
    for blk in range(4):
        ltv = np.zeros((NPAIR, 128, LTW), np.float64)
        ltv8 = np.zeros((NPAIR, 128, LT8W), np.float64)
        for p in range(NPAIR):
            for r in blocks[blk]['pairs'][p]['records']:
                if r['mode'] == 'f16':
                    fill_em(ltv[p], r['ltcol'], r['parts'][0], r['width'])
                else:
                    a, bb = r['parts']
                    fill_em(ltv8[p], r['ltcol'], a, r['width'])
                    fill_em(ltv8[p], r['ltcol'] + r['width'], bb,
                            r['width'])
        lt_blk.append(ltv.astype(np.float16))
        lt8_blk.append(ltv8.astype(np.float16).astype(f8d))

    in_maps = []
    for cid in range(NCORES):
        b, blk = cid // 4, cid % 4
        xz = np.concatenate([x[b], np.zeros((C, 1, W), x.dtype)], axis=1)
        xz = xz.astype(np.float16)
        rows = np.asarray(blocks[blk]['events'], np.int64)
        pair_idx = np.stack([rows, rows + 1], axis=1)       # [E, 2]
        xbv = xz[:, pair_idx, :]                            # [C, E, 2, W]
        xbv = np.ascontiguousarray(xbv.transpose(1, 2, 0, 3))  # [E,2,C,W]
        in_maps.append({
            'xb': xbv,
            'xb8': np.ascontiguousarray(xbv.astype(f8d)),
            'lt': lt_blk[blk],
            'lt8': lt8_blk[blk],
            'lts': lts_on if blk == 0 else lts_off,
            'blkid': np.array([[blk]], np.int32),
            'coefr': coef_on if blk == 0 else coef_off,
            'biasd': biasd,
        })
    return in_maps


def _gather(res):
    """Assemble full output from per-core results (iteration order -> rows)."""
    blocks = _CACHE['prog'][2]
    out = np.empty((B, O, H, W), np.float32)
    for cid in range(NCORES):
        b, blk = cid // 4, cid % 4
        oc = res.results[cid]['out']                        # [NPAIR, 128, W]
        hps = np.array([pr['hpair']
                        for pr in blocks[blk]['pairs']], np.int64)
        rows0 = blk * NROW + 2 * hps
        out[b, :, rows0, :] = oc[:, 0:64, :]
        out[b, :, rows0 + 1, :] = oc[:, 64:128, :]
    return out


def kernel(x, weight, bias):
    from concourse.bass_utils import run_bass_kernel_spmd
    x = np.asarray(x, dtype=np.float32)
    weight = np.asarray(weight, dtype=np.float32)
    bias = np.asarray(bias, dtype=np.float32)

    nc, tt, blocks, E, LTW = _get_compiled()
    in_maps = _core_inputs(x, weight, bias, tt, blocks, E, LTW)
    res = run_bass_kernel_spmd(nc, in_maps, core_ids=list(range(NCORES)))
    return _gather(res)
